# revision 19
# baseline (speedup 1.0000x reference)
"""Bass/Trainium2 kernel for nn_GAT_56246891708452 (4-layer GAT-style GNN).

Strategy: graph-data-parallel across 8 NeuronCores (25 graphs each). Inside a
core everything lives in SBUF feature-major ([feat, entity]); edges are packed
two-halves-deep ([128, E/2]). ELU is computed as the pair
(t, r) = (min(exp(x),1), relu(x)) with elu(x) = t + r - 1; the "-1" and all
BatchNorm affines / inner biases are folded into downstream weights on the
host, so each MLP block is exactly: matmuls into PSUM + one ACT evacuation.
x[dst] gathers run on the tensor engine as per-graph one-hot matmuls against a
node-major copy of (Wd @ x) produced by PE transposes.
"""

import numpy as np
import ml_dtypes
from contextlib import ExitStack

import concourse.bass as bass
import concourse.bacc as bacc
import concourse.tile as tile
import concourse.mybir as mybir
from concourse.bass_utils import run_bass_kernel_spmd

dt = mybir.dt
AF = mybir.ActivationFunctionType
ALU = mybir.AluOpType

F = 64          # feature width
GPN = 100       # nodes per graph
DEG = 8         # edges per node
NLAYERS = 4
EPS = 1e-5
NEG = 0.2
BNI = 1.0 / np.sqrt(1.0 + EPS)

BF = ml_dtypes.bfloat16


# ----------------------------------------------------------------------------
# Host-side weight folding
# ----------------------------------------------------------------------------
def _fold_block(p):
    """Return (W', b') with BN affine folded: block(z) = W' @ z + b'."""
    W = np.asarray(p["W"], np.float32)
    b = np.asarray(p["b"], np.float32)
    g = np.asarray(p["g"], np.float32)
    be = np.asarray(p["be"], np.float32)
    s = g * BNI
    return W * s[:, None], b * s + be


def _prep_weights(params):
    """Build the packed per-layer weight/bias arrays (shared by all cores)."""
    shared = {}
    for l, lp in enumerate(np.asarray(params["layers"], dtype=object) if isinstance(params["layers"], list) is False else params["layers"]):
        EM1W, EM1b = _fold_block(lp["edge_model"][0])
        EM2W, EM2b = _fold_block(lp["edge_model"][1])
        ML1W, ML1b = _fold_block(lp["edge_mlp"][0])
        ML2W, ML2b = _fold_block(lp["edge_mlp"][1])
        Wd, Ws, Wea, Wu = EM1W[:, 0:64], EM1W[:, 64:128], EM1W[:, 128:192], EM1W[:, 192:256]
        W32 = ML1W[:, 64:128] @ EM2W
        b3 = ML1b + ML1W[:, 64:128] @ EM2b - W32.sum(1)        # elu "-1" fold
        b4 = ML2b - ML2W.sum(1)                                # elu "-1" fold

        # edge weights, bf16, packed into one [128, 384] array of lhsT tiles
        EW = np.zeros((128, 384), np.float32)
        EW[0:64, 0:64] = Ws.T;  EW[64:128, 0:64] = Wu.T        # WsuT (K=128)
        EW[0:128, 64:128] = np.vstack([W32.T, W32.T])          # W32pair
        EW[0:128, 128:192] = np.vstack([ML2W.T, ML2W.T])       # W4pair
        EW[0:64, 192:256] = Wea.T;  EW[64:128, 192:256] = Wea.T
        EW[0:64, 256:320] = Wd.T
        EW[0:64, 320:384] = ML1W[:, 0:64].T                    # W3a
        EW[64:128, 320:384] = ML1W[:, 0:64].T
        shared[f"EW{l}"] = EW.astype(BF)

        # node/global weights, f32, packed [128, 840]
        NM1W, NM1b = _fold_block(lp["node_model"][0])
        NM2W, NM2b = _fold_block(lp["node_model"][1])
        NP1W, NP1b = _fold_block(lp["node_mlp"][0])
        NP2W, NP2b = _fold_block(lp["node_mlp"][1])
        W42 = NP1W[:, 64:128] @ NM2W
        nb2 = NP1b + NP1W[:, 64:128] @ NM2b - W42.sum(1)
        nb3 = NP2b - NP2W.sum(1)
        G1W, G1b = _fold_block(lp["global_model"][0])
        G2W, G2b = _fold_block(lp["global_model"][1])
        G3W, G3b = _fold_block(lp["global_model"][2])
        M1W, M1b = _fold_block(lp["global_mlp"][0])
        M2W, M2b = _fold_block(lp["global_mlp"][1])
        gb2 = G2b - G2W.sum(1)
        gb3 = G3b - G3W.sum(1)
        mb2 = M2b - M2W.sum(1)
        a = np.asarray(lp["att_global"], np.float32)[0, 0]

        NW = np.zeros((128, 840), np.float32)
        NW[0:64, 0:64] = NM1W[:, 0:64].T                       # Wx
        NW[64:128, 0:64] = NM1W[:, 64:128].T                   # Wu  (K=128 pair)
        NW[0:128, 64:128] = np.vstack([W42.T, W42.T])
        NW[0:128, 128:192] = np.vstack([NP2W.T, NP2W.T])       # x_new pair
        NW[0:128, 192:256] = np.vstack([G2W.T, G2W.T])
        NW[0:128, 256:320] = np.vstack([G3W.T, G3W.T])
        NW[0:128, 320:384] = np.vstack([M2W.T, M2W.T])
        NW[0:64, 384:448] = NM1W[:, 128:192].T                 # Wagg
        NW[64:128, 384:448] = NM1W[:, 128:192].T
        NW[0:64, 448:512] = NP1W[:, 0:64].T                    # Wpx
        NW[0:64, 512:576] = G1W[:, 0:64].T                     # G1u
        NW[0:64, 576:640] = G1W[:, 64:128].T                   # G1o
        NW[0:64, 640:704] = M1W[:, 0:64].T                     # M1u
        NW[0:64, 704:768] = M1W[:, 64:128].T                   # M1o
        NW[0:64, 768:770] = np.stack([a[:64], a[64:]], 1)      # attT
        NW[:, 770:834] = 1.0                                   # ones rows (any base)
        shared[f"NW{l}"] = NW

        BIAS = np.zeros((128, 16), np.float32)
        BIAS[:, 0] = np.concatenate([EM1b, EM1b])
        BIAS[:, 1] = np.concatenate([b3, b3])
        BIAS[:, 2] = np.concatenate([b4, b4])
        BIAS[0:64, 3] = NM1b
        BIAS[0:64, 4] = nb2
        BIAS[0:64, 5] = nb3
        BIAS[0:64, 6] = G1b
        BIAS[0:64, 7] = gb2
        BIAS[0:64, 8] = gb3
        BIAS[0:64, 9] = M1b
        BIAS[0:64, 10] = mb2
        shared[f"BIAS{l}"] = BIAS

    # classifier
    c = params["clf"]
    s = np.asarray(c["g"], np.float32) * BNI
    C1W = np.asarray(c["W1"], np.float32) * s[:, None]
    C1b = np.asarray(c["b1"], np.float32) * s + np.asarray(c["be"], np.float32)
    W2 = np.asarray(c["W2"], np.float32)
    b2 = np.asarray(c["b2"], np.float32)
    wfin = W2[1] - W2[0]
    bfin = (b2[1] - b2[0]) - wfin.sum()                        # elu "-1" fold
    C1T = np.zeros((64, 512), np.float32)
    for k in range(8):
        C1T[:, 64 * k:64 * (k + 1)] = C1W[:, 64 * k:64 * (k + 1)].T
    shared["C1T"] = C1T
    shared["WFIN"] = np.concatenate([wfin, wfin]).reshape(128, 1).astype(np.float32)
    CB = np.zeros((128, 2), np.float32)
    CB[:, 0] = np.concatenate([C1b, C1b])
    CB[0:1, 1] = bfin
    shared["CBIAS"] = CB
    shared["IDENT"] = np.eye(64, dtype=np.float32).astype(BF)
    return shared


def _prep_core(inputs, params, c, G):
    """Per-core data arrays for core c owning graphs [G*c, G*(c+1))."""
    NL = G * GPN
    EL = NL * DEG
    EH = EL // 2
    n0, n1 = c * NL, (c + 1) * NL
    e0, e1 = n0 * DEG, n1 * DEG

    x = np.asarray(inputs["x"], np.float32)[n0:n1]             # [NL, F]
    ea = np.asarray(inputs["edge_attr"], np.float32)[e0:e1]    # [EL, F]
    dij = np.asarray(inputs["dij"], np.float32)[e0:e1, 0]      # [EL]
    u = np.asarray(inputs["u"], np.float32)[c * G:(c + 1) * G]  # [G, F]
    Pij = np.asarray(inputs["Pij"], np.float32)[n0:n1]         # [NL, 3]
    wij = np.asarray(inputs["wij"], np.float32)[n0:n1, 0]      # [NL]
    dst = np.asarray(inputs["edge_index"])[1, e0:e1].astype(np.int64) - n0

    def half_pack(arrT):                                       # [F, EL] -> [128, EH]
        return np.concatenate([arrT[:, 0:EH], arrT[:, EH:EL]], axis=0)

    eaT = ea.T.copy()
    core = {
        "ea0": half_pack(eaT).astype(BF),
        "eadij0": half_pack(eaT * dij[None, :]).astype(BF),
        "dijrep": half_pack(np.broadcast_to(dij[None, :], (F, EL))).astype(BF),
        "x0": x.T.copy().astype(np.float32),                   # [64, NL]
        "u0": u.T.copy().astype(np.float32),                   # [64, G]
    }
    # one-hot gather matrix [100, EL] (cols in global edge order)
    oh = np.zeros((GPN, EL), np.float32)
    loc = dst % GPN
    oh[loc, np.arange(EL)] = 1.0
    core["onehot"] = oh.astype(BF)

    # layer-0 node-major (Wd0 @ x).T per graph: [100, 64*G]
    lp0 = params["layers"][0]
    EM1W, _ = _fold_block(lp0["edge_model"][0])
    Wd0 = EM1W[:, 0:64]
    xw0 = np.zeros((GPN, F * G), np.float32)
    for g in range(G):
        xw0[:, F * g:F * (g + 1)] = x[g * GPN:(g + 1) * GPN] @ Wd0.T
    core["xw0"] = xw0.astype(BF)

    # host-computed Pij features (pf) and (wij + pw_softmax)/3 per layer
    def elu(v):
        return np.where(v > 0, v, np.exp(np.minimum(v, 0)) - 1)

    pf_all = np.zeros((F, NLAYERS * NL), np.float32)
    wpw_all = np.zeros((G, NLAYERS * GPN), np.float32)
    for l, lp in enumerate(params["layers"]):
        Q1W, Q1b = _fold_block(lp["Pij_model"][0])
        Q2W, Q2b = _fold_block(lp["Pij_model"][1])
        pf = Q2W @ elu(Q1W @ Pij.T + Q1b[:, None]) + Q2b[:, None]
        pf_all[:, l * NL:(l + 1) * NL] = pf
        pijw = np.asarray(lp["pij"], np.float32)[0, 0]
        pw = Pij @ pijw                                        # [NL]
        pw = np.where(pw > 0, pw, NEG * pw).reshape(G, GPN)
        pw = pw - pw.max(1, keepdims=True)
        e = np.exp(pw)
        pw = e / e.sum(1, keepdims=True)
        wpw_all[:, l * GPN:(l + 1) * GPN] = (wij.reshape(G, GPN) + pw) / 3.0
    core["PF"] = pf_all
    core["WPW"] = wpw_all
    return core


# ----------------------------------------------------------------------------
# Bass program
# ----------------------------------------------------------------------------
def _build(G):
    NL = G * GPN
    EL = NL * DEG
    EH = EL // 2
    NH = NL // 2
    EPG = GPN * DEG   # edges per graph = 800
    f32, f32r, bf16 = dt.float32, dt.float32r, dt.bfloat16

    nc = bacc.Bacc("TRN2", target_bir_lowering=False)

    D = {}
    def dparam(name, shape, dtype):
        D[name] = nc.dram_tensor(name, list(shape), dtype, kind="ExternalInput")
        return D[name]

    for l in range(NLAYERS):
        dparam(f"EW{l}", (128, 384), bf16)
        dparam(f"NW{l}", (128, 840), f32r)
        dparam(f"BIAS{l}", (128, 16), f32)
    dparam("C1T", (64, 512), f32)
    dparam("WFIN", (128, 1), f32)
    dparam("CBIAS", (128, 2), f32)
    dparam("IDENT", (64, 64), bf16)
    dparam("ea0", (128, EH), bf16)
    dparam("eadij0", (128, EH), bf16)
    dparam("dijrep", (128, EH), bf16)
    dparam("onehot", (GPN, EL), bf16)
    dparam("xw0", (GPN, F * G), bf16)
    dparam("x0", (64, NL), f32r)
    dparam("u0", (64, G), f32)
    dparam("PF", (64, NLAYERS * NL), f32r)
    dparam("WPW", (G, NLAYERS * GPN), f32r)
    OUT = nc.dram_tensor("OUT", [1, G], f32, kind="ExternalOutput")
    att_dram = nc.dram_tensor("att_scr", [2, NL], f32)
    wg_dram = nc.dram_tensor("wg_scr", [G, GPN], f32r)

    SB = min(1024, EH)          # superblock width (cols per half)
    nsb = (EH + SB - 1) // SB

    with tile.TileContext(nc) as tc, ExitStack() as ctx:
        ctx.enter_context(nc.allow_low_precision(reason="float32r accumulators are 32-bit"))
        P = ctx.enter_context(tc.tile_pool(name="pers", bufs=1))

        # ---- persistent SBUF state ----
        ew = [P.tile([128, 384], bf16, tag=f"ew{l}", name=f"ew{l}") for l in range(NLAYERS)]
        nw = [P.tile([128, 840], f32r, tag=f"nw{l}", name=f"nw{l}") for l in range(NLAYERS)]
        bia = [P.tile([128, 16], f32, tag=f"bias{l}", name=f"bias{l}") for l in range(NLAYERS)]
        for l in range(NLAYERS):
            nc.sync.dma_start(ew[l][:], D[f"EW{l}"][:])
            nc.sync.dma_start(nw[l][:], D[f"NW{l}"][:])
            nc.sync.dma_start(bia[l][:], D[f"BIAS{l}"][:])
        c1t = P.tile([64, 512], f32, tag="c1t"); nc.sync.dma_start(c1t[:], D["C1T"][:])
        wfin = P.tile([128, 1], f32, tag="wfin"); nc.sync.dma_start(wfin[:], D["WFIN"][:])
        cbias = P.tile([128, 2], f32, tag="cbias"); nc.sync.dma_start(cbias[:], D["CBIAS"][:])
        ident = P.tile([64, 64], bf16, tag="ident"); nc.sync.dma_start(ident[:], D["IDENT"][:])

        ea = P.tile([128, EH], bf16, tag="ea"); nc.sync.dma_start(ea[:], D["ea0"][:])
        eadij = P.tile([128, EH], bf16, tag="eadij"); nc.sync.dma_start(eadij[:], D["eadij0"][:])
        dijrep = P.tile([128, EH], bf16, tag="dijrep"); nc.sync.dma_start(dijrep[:], D["dijrep"][:])
        onehot = P.tile([GPN, EL], bf16, tag="onehot"); nc.sync.dma_start(onehot[:], D["onehot"][:])
        xw_nm = P.tile([GPN, F * G], bf16, tag="xw_nm"); nc.sync.dma_start(xw_nm[:], D["xw0"][:])
        xuF = P.tile([128, NL], f32r, tag="xuF")
        nc.sync.dma_start(xuF[0:64, :], D["x0"][:])
        u_init = P.tile([64, G], f32, tag="u_init"); nc.sync.dma_start(u_init[:], D["u0"][:])
        wpw = P.tile([G, NLAYERS * GPN], f32r, tag="wpw"); nc.sync.dma_start(wpw[:], D["WPW"][:])
        xu_bf = P.tile([128, NL], bf16, tag="xu_bf")
        agg2 = P.tile([128, NH], f32r, tag="agg2")
        xs_save = P.tile([64, NLAYERS * G], f32, tag="xs_save")
        us_save = P.tile([64, NLAYERS * G], f32, tag="us_save")

        pfp = ctx.enter_context(tc.tile_pool(name="pf", bufs=1))
        c1p = ctx.enter_context(tc.tile_pool(name="c1bf", bufs=1))

        for l in range(NLAYERS):
            EWl, NWl, Bl = ew[l], nw[l], bia[l]
            WsuT = EWl[:, 0:64]; W32p = EWl[:, 64:128]; W4p = EWl[:, 128:192]
            WeaT = EWl[0:64, 192:256]; WdT = EWl[0:64, 256:320]; W3aT = EWl[0:64, 320:384]
            b1ap = Bl[:, 0:1]; b3ap = Bl[:, 1:2]; b4ap = Bl[:, 2:3]

            pf_l = pfp.tile([64, NL], f32r, tag="pf")
            nc.sync.dma_start(pf_l[:], D["PF"][:, l * NL:(l + 1) * NL])

            # ---- A/B: broadcast u into xuF rows 64:128, cast both to bf16 ----
            u_src = u_init[:, :] if l == 0 else us_save[:, (l - 1) * G:l * G]
            nc.vector.tensor_copy(
                xuF[64:128, :].rearrange("p (a b) -> p a b", b=GPN),
                u_src.unsqueeze(2).broadcast_to([64, G, GPN]))
            nc.scalar.activation(xu_bf[:], xuF[:], AF.Identity, bias=0.0)

            # ---- C: node-major (Wd @ x).T per graph (layers >= 1) ----
            if l > 0:
                with tc.tile_pool(name="psC", bufs=1, space="PSUM") as psC, \
                     tc.tile_pool(name="psT", bufs=2, space="PSUM") as psT, \
                     tc.tile_pool(name="sbT", bufs=1) as sbT:
                    c1ps = psC.tile([64, NL], f32)
                    for c0 in range(0, NL, 512):
                        c1 = min(c0 + 512, NL)
                        nc.tensor.matmul(c1ps[:, c0:c1], WdT, xu_bf[0:64, c0:c1],
                                         start=True, stop=True)
                    c1bf = sbT.tile([64, NL], bf16, tag="c1bf")
                    nc.scalar.activation(c1bf[:], c1ps[:], AF.Identity, bias=0.0)
                    for g0 in range(0, G, 2):
                        gw = min(2, G - g0)
                        pt = psT.tile([GPN, 128], bf16, tag="pt")
                        for gg in range(gw):
                            g = g0 + gg
                            nc.tensor.transpose(pt[:, 64 * gg:64 * (gg + 1)],
                                                c1bf[:, GPN * g:GPN * (g + 1)], ident[:])
                        nc.vector.tensor_copy(xw_nm[:, 64 * g0:64 * (g0 + gw)],
                                              pt[:, 0:64 * gw])

            # ---- D: edge superblocks ----
            with tc.tile_pool(name="ps1", bufs=2, space="PSUM") as ps1, \
                 tc.tile_pool(name="ps2", bufs=1, space="PSUM") as ps2, \
                 tc.tile_pool(name="ps3", bufs=1, space="PSUM") as ps3, \
                 tc.tile_pool(name="sbE", bufs=2) as sbE:
                for sb in range(nsb):
                    a0 = SB * sb
                    a1 = min(a0 + SB, EH)
                    W = a1 - a0
                    P1 = ps1.tile([128, W], f32, tag="P1")
                    for h in range(2):
                        tp = None if h == 0 else (0, 64)
                        tpk = None if h == 0 else (64, 64)
                        orow = slice(64 * h, 64 * h + 64)
                        for c0 in range(a0, a1, 512):
                            c1 = min(c0 + 512, a1)
                            ge0 = h * EH + c0          # global edge id range
                            ge1 = h * EH + c1
                            # gather one-hot segments (per graph)
                            s = ge0
                            first = True
                            while s < ge1:
                                g = s // EPG
                                e = min(ge1, (g + 1) * EPG)
                                nc.tensor.matmul(
                                    P1[orow, s - ge0 + c0 - a0:e - ge0 + c0 - a0],
                                    xw_nm[:, F * g:F * (g + 1)],
                                    onehot[:, s:e],
                                    start=first and True, stop=False,
                                    tile_position=tp, skip_group_check=True)
                                first = False
                                s = e
                            # [x;u] source-broadcast chunk (K=128)
                            n0 = ge0 // DEG
                            nw_ = (c1 - c0) // DEG
                            nc.tensor.matmul(
                                P1[orow, c0 - a0:c1 - a0],
                                WsuT,
                                xu_bf[:, n0:n0 + nw_].unsqueeze(2)
                                    .broadcast_to([128, nw_, DEG]),
                                start=False, stop=False,
                                tile_position=tp, skip_group_check=True)
                            # ea*dij chunk (K=64)
                            nc.tensor.matmul(
                                P1[orow, c0 - a0:c1 - a0],
                                EWl[orow, 192:256], eadij[orow, c0:c1],
                                start=False, stop=True,
                                tile_position=tpk, skip_group_check=True)
                    E1 = sbE.tile([128, W], bf16, tag="E1")
                    nc.scalar.activation(E1[:], P1[:], AF.Exp, bias=b1ap)
                    M1 = sbE.tile([128, W], bf16, tag="M1")
                    nc.vector.tensor_scalar(M1[:], E1[:], 1.0, None, ALU.min)
                    R1 = sbE.tile([128, W], bf16, tag="R1")
                    nc.vector.tensor_scalar(R1[:], P1[:], b1ap, 0.0, ALU.add, ALU.max)

                    P2 = ps2.tile([128, W], f32, tag="P2")
                    for h in range(2):
                        tpk = None if h == 0 else (64, 64)
                        orow = slice(64 * h, 64 * h + 64)
                        for c0 in range(0, W, 512):
                            c1 = min(c0 + 512, W)
                            cc = slice(c0, c1)
                            nc.tensor.matmul(P2[orow, cc], EWl[orow, 320:384], ea[orow, a0 + c0:a0 + c1],
                                             start=True, stop=False,
                                             tile_position=tpk, skip_group_check=True)
                            nc.tensor.matmul(P2[orow, cc], EWl[orow, 64:128], M1[orow, cc],
                                             start=False, stop=False,
                                             tile_position=tpk, skip_group_check=True)
                            nc.tensor.matmul(P2[orow, cc], EWl[orow, 64:128], R1[orow, cc],
                                             start=False, stop=True,
                                             tile_position=tpk, skip_group_check=True)
                    E3 = sbE.tile([128, W], bf16, tag="E1")
                    nc.scalar.activation(E3[:], P2[:], AF.Exp, bias=b3ap)
                    M3 = sbE.tile([128, W], bf16, tag="M1")
                    nc.vector.tensor_scalar(M3[:], E3[:], 1.0, None, ALU.min)
                    R3 = sbE.tile([128, W], bf16, tag="R1")
                    nc.vector.tensor_scalar(R3[:], P2[:], b3ap, 0.0, ALU.add, ALU.max)

                    P3 = ps3.tile([128, W], f32, tag="P3")
                    for h in range(2):
                        tpk = None if h == 0 else (64, 64)
                        orow = slice(64 * h, 64 * h + 64)
                        for c0 in range(0, W, 512):
                            c1 = min(c0 + 512, W)
                            cc = slice(c0, c1)
                            nc.tensor.matmul(P3[orow, cc], EWl[orow, 128:192], M3[orow, cc],
                                             start=True, stop=False,
                                             tile_position=tpk, skip_group_check=True)
                            nc.tensor.matmul(P3[orow, cc], EWl[orow, 128:192], R3[orow, cc],
                                             start=False, stop=True,
                                             tile_position=tpk, skip_group_check=True)
                    nc.scalar.activation(ea[:, a0:a1], P3[:], AF.Identity, bias=b4ap)
                    nc.vector.tensor_tensor(eadij[:, a0:a1], ea[:, a0:a1],
                                            dijrep[:, a0:a1], ALU.mult)

            # ---- E/F: agg + node chain ----
            with tc.tile_pool(name="psN", bufs=1, space="PSUM") as psN, \
                 tc.tile_pool(name="psG", bufs=2, space="PSUM") as psG, \
                 tc.tile_pool(name="sbN", bufs=1) as sbN:
                nc.vector.tensor_reduce(
                    agg2[:], eadij[:].rearrange("p (n k) -> p n k", k=DEG),
                    mybir.AxisListType.X, ALU.add)

                Pn = psN.tile([64, NL], f32, tag="np")
                for c0 in range(0, NL, 512):
                    c1 = min(c0 + 512, NL)
                    nc.tensor.matmul(Pn[:, c0:c1], NWl[:, 0:64], xuF[:, c0:c1],
                                     start=True, stop=False, skip_group_check=True)
                # agg chunks (row-half of agg2 by node range); segments must stay
                # inside one psum bank AND one agg2 row-half
                cuts = sorted(set(list(range(0, NL, 512)) + [NH, NL]))
                segs = [(a, b) for a, b in zip(cuts[:-1], cuts[1:])]
                for i, (s0, s1) in enumerate(segs):
                    h = 0 if s0 < NH else 1
                    base = h * NH
                    nc.tensor.matmul(Pn[:, s0:s1],
                                     NWl[64 * h:64 * h + 64, 384:448],
                                     agg2[64 * h:64 * h + 64, s0 - base:s1 - base],
                                     start=False, stop=(i == len(segs) - 1),
                                     skip_group_check=True)

                def node_elu(ps, bias_ap):
                    En = sbN.tile([64, NL], f32, tag="En")
                    nc.scalar.activation(En[:], ps[:], AF.Exp, bias=bias_ap)
                    TR = sbN.tile([128, NL], f32r, tag="TR")
                    nc.vector.tensor_scalar(TR[0:64, :], En[:], 1.0, None, ALU.min)
                    nc.vector.tensor_scalar(TR[64:128, :], ps[:], bias_ap, 0.0,
                                            ALU.add, ALU.max)
                    return TR

                TRn = node_elu(Pn, Bl[0:64, 3:4])
                Pn2 = psN.tile([64, NL], f32, tag="np")
                for c0 in range(0, NL, 512):
                    c1 = min(c0 + 512, NL)
                    nc.tensor.matmul(Pn2[:, c0:c1], NWl[:, 64:128], TRn[:, c0:c1],
                                     start=True, stop=False, skip_group_check=True)
                    nc.tensor.matmul(Pn2[:, c0:c1], NWl[0:64, 448:512], xuF[0:64, c0:c1],
                                     start=False, stop=True, skip_group_check=True)
                TRn2 = node_elu(Pn2, Bl[0:64, 4:5])
                Pn3 = psN.tile([64, NL], f32, tag="np")
                for c0 in range(0, NL, 512):
                    c1 = min(c0 + 512, NL)
                    nc.tensor.matmul(Pn3[:, c0:c1], NWl[:, 128:192], TRn2[:, c0:c1],
                                     start=True, stop=True, skip_group_check=True)
                nc.scalar.activation(xuF[0:64, :], Pn3[:], AF.Identity, bias=Bl[0:64, 5:6])
                nc.vector.tensor_copy(xs_save[:, l * G:(l + 1) * G], xuF[0:64, 0:NL:GPN])

                # ---- G: attention -> w ----
                Pa = psN.tile([2, NL], f32, tag="np")
                for c0 in range(0, NL, 512):
                    c1 = min(c0 + 512, NL)
                    nc.tensor.matmul(Pa[:, c0:c1], NWl[0:64, 768:770], xuF[0:64, c0:c1],
                                     start=True, stop=True, skip_group_check=True)
                att2 = sbN.tile([2, NL], f32, tag="att2")
                nc.scalar.activation(att2[:], Pa[:], AF.Identity, bias=0.0)
                nc.sync.dma_start(att_dram[:], att2[:])
                attg = sbN.tile([G, GPN], f32, tag="attg")
                nc.sync.dma_start(attg[:], att_dram[1:2, :].rearrange("p (g n) -> (p g) n", g=G))
                att0 = sbN.tile([G, 1], f32, tag="att0")
                nc.sync.dma_start(att0[:], att_dram[0:1, 0:NL:GPN].rearrange("p g -> (p g)").unsqueeze(1))
                nc.vector.tensor_tensor(attg[:], attg[:], att0[:].broadcast_to([G, GPN]),
                                        ALU.add)
                lrt = sbN.tile([G, GPN], f32, tag="lrt")
                nc.vector.tensor_scalar(lrt[:], attg[:], NEG, None, ALU.mult)
                nc.vector.tensor_tensor(attg[:], attg[:], lrt[:], ALU.max)
                mx = sbN.tile([G, 1], f32, tag="mx")
                nc.vector.tensor_reduce(mx[:], attg[:], mybir.AxisListType.X, ALU.max,
                                        negate=True)
                eg = sbN.tile([G, GPN], f32, tag="eg")
                nc.scalar.activation(eg[:], attg[:], AF.Exp, bias=mx[:])
                sm = sbN.tile([G, 1], f32, tag="sm")
                nc.vector.tensor_reduce(sm[:], eg[:], mybir.AxisListType.X, ALU.add)
                nc.vector.tensor_scalar(sm[:], sm[:], 3.0, None, ALU.mult)
                rs = sbN.tile([G, 1], f32, tag="rs")
                nc.vector.reciprocal(rs[:], sm[:])
                wg = sbN.tile([G, GPN], f32r, tag="wg")
                nc.vector.tensor_tensor(wg[:], eg[:], rs[:].broadcast_to([G, GPN]), ALU.mult)
                nc.vector.tensor_tensor(wg[:], wg[:], wpw[:, l * GPN:(l + 1) * GPN], ALU.add)

                # ---- H: u_out + global chain ----
                nc.sync.dma_start(wg_dram[:], wg[:])
                wg1 = sbN.tile([1, NL], f32r, tag="wg1")
                nc.sync.dma_start(wg1[:], wg_dram[:].rearrange("g n -> (g n)").unsqueeze(0))
                Pw = psN.tile([64, NL], f32, tag="np")
                for c0 in range(0, NL, 512):
                    c1 = min(c0 + 512, NL)
                    nc.tensor.matmul(Pw[:, c0:c1], NWl[0:1, 770:834], wg1[0:1, c0:c1],
                                     start=True, stop=True, skip_group_check=True)
                pxw = sbN.tile([64, NL], f32, tag="En")
                nc.vector.tensor_tensor(pxw[:], pf_l[:], xuF[0:64, :], ALU.mult)
                nc.vector.tensor_tensor(pxw[:], pxw[:], Pw[:], ALU.mult)
                u_out = sbN.tile([64, G], f32, tag="uo")
                nc.vector.tensor_reduce(u_out[:], pxw[:].rearrange("p (g n) -> p g n", n=GPN),
                                        mybir.AxisListType.X, ALU.add)

                ucur = u_init[:, :] if l == 0 else us_save[:, (l - 1) * G:l * G]
                def gmm2(lhs_list, rhs_list, start_stop=True):
                    pg = psG.tile([64, G], f32, tag="pg")
                    n = len(lhs_list)
                    for i, (lh, rh) in enumerate(zip(lhs_list, rhs_list)):
                        nc.tensor.matmul(pg[:], lh.bitcast(dt.float32), rh,
                                         start=(i == 0), stop=(i == n - 1),
                                         skip_group_check=True)
                    return pg

                def small_elu(ps, bias_ap):
                    e = sbN.tile([64, G], f32, tag="se")
                    nc.scalar.activation(e[:], ps[:], AF.Exp, bias=bias_ap)
                    tr = sbN.tile([128, G], f32, tag="str")
                    nc.vector.tensor_scalar(tr[0:64, :], e[:], 1.0, None, ALU.min)
                    nc.vector.tensor_scalar(tr[64:128, :], ps[:], bias_ap, 0.0,
                                            ALU.add, ALU.max)
                    return tr

                g1 = gmm2([NWl[0:64, 512:576], NWl[0:64, 576:640]], [ucur, u_out[:]])
                tg1 = small_elu(g1, Bl[0:64, 6:7])
                g2 = gmm2([NWl[:, 192:256]], [tg1[:]])
                tg2 = small_elu(g2, Bl[0:64, 7:8])
                g3 = gmm2([NWl[:, 256:320]], [tg2[:]])
                uo2 = sbN.tile([64, G], f32, tag="uo2")
                nc.scalar.activation(uo2[:], g3[:], AF.Identity, bias=Bl[0:64, 8:9])
                m1 = gmm2([NWl[0:64, 640:704], NWl[0:64, 704:768]], [ucur, uo2[:]])
                tm1 = small_elu(m1, Bl[0:64, 9:10])
                m2 = gmm2([NWl[:, 320:384]], [tm1[:]])
                nc.scalar.activation(us_save[:, l * G:(l + 1) * G], m2[:],
                                     AF.Identity, bias=Bl[0:64, 10:11])

        # ---- classifier ----
        with tc.tile_pool(name="psF", bufs=2, space="PSUM") as psF, \
             tc.tile_pool(name="sbF", bufs=1) as sbF:
            pc = psF.tile([64, G], f32, tag="pc")
            for k in range(8):
                rhs = xs_save[:, (k % 4) * G:(k % 4 + 1) * G] if k < 4 \
                    else us_save[:, (k - 4) * G:(k - 3) * G]
                nc.tensor.matmul(pc[:], c1t[:, 64 * k:64 * (k + 1)], rhs,
                                 start=(k == 0), stop=(k == 7), skip_group_check=True)
            ec = sbF.tile([64, G], f32, tag="ec")
            nc.scalar.activation(ec[:], pc[:], AF.Exp, bias=cbias[0:64, 0:1])
            trc = sbF.tile([128, G], f32, tag="trc")
            nc.vector.tensor_scalar(trc[0:64, :], ec[:], 1.0, None, ALU.min)
            nc.vector.tensor_scalar(trc[64:128, :], pc[:], cbias[0:64, 0:1], 0.0,
                                    ALU.add, ALU.max)
            pfin = psF.tile([1, G], f32, tag="pfin")
            nc.tensor.matmul(pfin[:], wfin[:], trc[:], start=True, stop=True,
                             skip_group_check=True)
            osb = sbF.tile([1, G], f32, tag="osb")
            nc.scalar.activation(osb[:], pfin[:], AF.Sigmoid, bias=cbias[0:1, 1:2])
            nc.sync.dma_start(OUT[:], osb[:])

    nc.compile()
    return nc


# ----------------------------------------------------------------------------
# Driver
# ----------------------------------------------------------------------------
def _prep_all(inputs, n_cores, G):
    params = inputs["params"]
    shared = _prep_weights(params)
    in_maps = []
    for c in range(n_cores):
        m = dict(shared)
        m.update(_prep_core(inputs, params, c, G))
        in_maps.append(m)
    return in_maps


def kernel(**inputs):
    n_cores = 8
    B = np.asarray(inputs["u"]).shape[0]
    G = B // n_cores
    in_maps = _prep_all(inputs, n_cores, G)
    nc = _build(G)
    res = run_bass_kernel_spmd(nc, in_maps, list(range(n_cores)))
    out = np.concatenate([res.results[c]["OUT"][0] for c in range(n_cores)])
    return out.astype(np.float32)
